# revision 31
# baseline (speedup 1.0000x reference)
"""Trainium2 Bass kernel for the SNN (snntorch Leaky, subtract-reset) forward.

Single fp32r matmul pass per step folds everything into one PE stream:
    P_t(h,b) accumulates  sum_s d_s*A(h,b) - sum_s c_s*spk_{s-1}
  where A = (W1 x + b1)/(1-beta), d_s = beta^-s (1-beta), c_s = beta^-s.
  Spike condition: mem_t > 1  <=>  P_t > tau_t = beta^-t  (scalar!).
Moving rows [128] = [spikes(100); x_hi(9); x_lo(9); x_hi strong8(8); 1; 1].
The x hi/lo/cross split delivers the theta decrement to ~22 bits despite
fp32r's ~11-bit ingest rounding. The spike subtract is exact via a two-
factor trick: the DVE compare writes spk in {0, K_t} (tensor_scalar
is_gt/mult) and the diag coefficient D_{t+1} is chosen so D_{t+1}*K_t ~
beta^-(t+1) to ~2^-22 (both factors 11-bit representable).
cur2 accumulates in PSUM rows 100:124 (6 pairs x 2 outs x hi/lo W2 split);
cumulative pair readouts after passes 7/13/19/25/26 are differenced in the
transposed post-phase, which also runs the cheap mem2 recurrence.
"""

import numpy as np

BETA = 0.95
T = 25
NPASS = 26
NI, NH, NO = 9, 100, 2
B = 65536
NCORES = 8
SH = B // NCORES          # 8192
RC = 2048
NR = SH // RC             # 4 rounds
NBLK = RC // 512
NJ = SH // 128            # 64 transpose blocks
f32 = np.float32
f64 = np.float64

_CACHE = {}
_LAST_RESULT = None
_LAST_IN_MAPS = None
_LAST_NC = None

RO_PASS = {7: 0, 13: 1, 19: 2, 25: 3, 26: 4}   # pass -> readout index


def _rnd11(v):
    """Round fp32 mantissa to 11 bits (half-up). Survives HW fp32r ingest."""
    v = np.asarray(v, f32)
    u = v.view(np.uint32).astype(np.uint64)
    u = (u + (np.uint64(1) << np.uint64(11))) >> np.uint64(12) << np.uint64(12)
    return (u & np.uint64(0xFFFFFFFF)).astype(np.uint32).view(f32)


def _find_DK(c):
    """11-bit pair (D, K) with D*K ~ c to ~2^-22."""
    best = (None, None, 1e9)
    for k in range(2048):
        K = 1.0 + k / 2048.0
        D = f64(_rnd11(f32(c / K)))
        e = abs(D * K - c)
        if e < best[2]:
            best = (D, K, e)
    return best[0], best[1]


def _host_consts(W1, b1, W2, b2):
    inv = 1.0 / (1.0 - f64(BETA))
    W1 = W1.astype(f64)
    b1 = b1.astype(f64)
    W2 = W2.astype(f64)
    b2 = b2.astype(f64)
    strong8 = np.argsort(-np.linalg.norm(W1, axis=0))[:8]

    # pair factorization: D_t * K_{t-1} ~ beta^-t  (t = 2..25)
    Kf = np.ones(T + 1, f64)          # Kf[t] scales spk_t (t = 1..25)
    Dg = np.zeros(NPASS + 1, f64)     # Dg[t] diag at pass t
    for t in range(2, T + 1):
        D, K = _find_DK(f64(BETA) ** -t)
        Dg[t] = D
        Kf[t - 1] = K
    tau = np.array([0.0] + [f32(f64(BETA) ** -t) for t in range(1, T + 1)],
                   f32)

    S = np.zeros((NPASS, 128, 124), f32)
    for t in range(1, NPASS + 1):
        s = S[t - 1]
        if 2 <= t:
            sp = t - 1                    # spike step consumed by this pass
            K = Kf[sp]
            if t <= T:
                np.fill_diagonal(s[0:NH, 0:NH], f32(-Dg[t]))
            p = (t - 2) % 6
            w2h = _rnd11((W2 / K).astype(f32))
            w2l = _rnd11(((W2 - w2h.astype(f64) * K) / K).astype(f32))
            for o in range(NO):
                s[0:NH, 100 + 2 * p + o] = w2h[o]
                s[0:NH, 112 + 2 * p + o] = w2l[o]
                s[126, 100 + 2 * p + o] = _rnd11(f32(b2[o]))
        if t <= T:
            d_t = f64(BETA) ** -t * (1.0 - f64(BETA))
            cf = d_t * W1 * inv                      # [NH, NI]
            chx = _rnd11(cf.astype(f32))
            clx = _rnd11((cf - chx.astype(f64)).astype(f32))
            ca = d_t * b1 * inv
            cah = _rnd11(ca.astype(f32))
            cal = _rnd11((ca - cah.astype(f64)).astype(f32))
            s[100:109, 0:NH] = chx.T
            s[109:118, 0:NH] = chx.T
            s[118:126, 0:NH] = clx.T[strong8]
            s[126, 0:NH] += cah
            s[127, 0:NH] = cal
    sm = S.transpose(1, 0, 2).reshape(128, NPASS * 124)
    return sm, tau, Kf.astype(f32), strong8


def _build_nc(tau, kf):
    import concourse.bass as bass
    import concourse.tile as tile
    from concourse import bacc, mybir

    f32d = mybir.dt.float32
    f32r = mybir.dt.float32r
    Copy = mybir.ActivationFunctionType.Copy
    Alu = mybir.AluOpType

    nc = bacc.Bacc("TRN2", target_bir_lowering=False, debug=False,
                   num_devices=NCORES)

    xt3_d = nc.dram_tensor("xt3", [28, SH], f32r, kind="ExternalInput").ap()
    sm_d = nc.dram_tensor("sm", [128, NPASS * 124], f32r,
                          kind="ExternalInput").ap()
    id_d = nc.dram_tensor("ident", [128, 128], f32d,
                          kind="ExternalInput").ap()
    out_d = nc.dram_tensor("out", [T, SH, NO], f32d,
                           kind="ExternalOutput").ap()

    with tile.TileContext(nc) as tc:
        with tc.tile_pool(name="const", bufs=1) as cp, \
             tc.tile_pool(name="ro", bufs=1) as rp:
            sm = cp.tile([128, NPASS * 124], f32r)
            ident = cp.tile([128, 128], f32d)
            # pass-1 slice first so the first matmuls start ~immediately
            nc.sync.dma_start(sm[:, 0:124], sm_d[:, 0:124])
            nc.sync.dma_start(sm[:, 124:3 * 124], sm_d[:, 124:3 * 124])
            nc.sync.dma_start(sm[:, 3 * 124:], sm_d[:, 3 * 124:])
            nc.sync.dma_start(ident[:], id_d[:])
            ro = rp.tile([120, SH], f32d)
            scr = [rp.tile([128, RC], f32d, name=f"scr{i}") for i in range(2)]

            with tc.tile_pool(name="spk", bufs=1) as kp:
                spks = [[kp.tile([128, RC], f32r, name=f"spk_{r}_{i}")
                         for i in range(2)] for r in range(NR)]
                for r in range(NR):
                    cs = slice(r * RC, (r + 1) * RC)
                    for i in range(2):
                        nc.sync.dma_start(spks[r][i][100:128, :],
                                          xt3_d[:, cs])
                    # pass 1 reads the parity-0 tile with zero spike coefs;
                    # garbage would still poison PSUM via 0*NaN
                    nc.vector.memset(spks[r][0][0:NH, :].bitcast(f32d), 0.0)

                for ph in range(2):
                    ps_pool = tc.tile_pool(name=f"ps{ph}", bufs=1,
                                           space=bass.MemorySpace.PSUM)
                    ps = ps_pool.__enter__()
                    rr = (2 * ph, 2 * ph + 1)
                    P = {r: ps.tile([128, RC], f32d, tag=f"P{r}",
                                    name=f"P_{r}") for r in rr}
                    for t in range(1, NPASS + 1):
                        st = sm[:, (t - 1) * 124:t * 124]
                        for r in rr:
                            mv = spks[r][(t - 1) % 2]
                            for b in range(NBLK):
                                bs = slice(b * 512, (b + 1) * 512)
                                nc.tensor.matmul(
                                    P[r][0:124, bs], st, mv[:, bs],
                                    start=(t == 1), stop=True,
                                    skip_group_check=True)
                            if t <= T:
                                nc.vector.tensor_scalar(
                                    spks[r][t % 2][0:NH, :], P[r][0:NH, :],
                                    float(tau[t]), float(kf[t]),
                                    Alu.is_gt, Alu.mult)
                            if t in RO_PASS:
                                k = RO_PASS[t]
                                sc = scr[r % 2]
                                nc.scalar.activation(sc[96:128, :],
                                                     P[r][96:128, :], Copy,
                                                     bias=0.0, scale=1.0)
                                nc.sync.dma_start(
                                    ro[24 * k:24 * k + 24,
                                       r * RC:(r + 1) * RC],
                                    sc[100:124, :])
                    ps_pool.__exit__(None, None, None)

            # ---- post: transpose, hi+lo, diff, mem2 recurrence, output ----
            with tc.tile_pool(name="post", bufs=1) as pp, \
                 tc.tile_pool(name="psT", bufs=1,
                              space=bass.MemorySpace.PSUM) as pt:
                tt = pp.tile([128, NJ, 120], f32d)
                for half in range(2):
                    # 128-col slots: each transpose output stays inside one
                    # 2KB PSUM bank (120-col slots would cross banks)
                    ptile = pt.tile([128, NJ // 2, 128], f32d, tag="tp",
                                    name=f"tp_{half}")
                    for jj in range(NJ // 2):
                        j = half * (NJ // 2) + jj
                        nc.tensor.transpose(
                            ptile[:, jj, 0:120],
                            ro[:, j * 128:(j + 1) * 128],
                            ident[0:120, 0:120])
                    nc.scalar.activation(
                        tt[:, half * (NJ // 2):(half + 1) * (NJ // 2), :],
                        ptile[:, :, 0:120], Copy, bias=0.0, scale=1.0)

                # s = hi + lo  -> [128, NJ, 60] (cols 12k + 2p + o)
                ssb = pp.tile([128, NJ, 60], f32d)
                tv = tt[:].rearrange("p j (k q) -> p (j k) q", q=24)
                sv = ssb[:].rearrange("p j (k q) -> p (j k) q", q=12)
                nc.vector.tensor_tensor(sv, tv[:, :, 0:12], tv[:, :, 12:24],
                                        Alu.add)
                # diff across consecutive readouts -> cur2 for t >= 7
                dsb = pp.tile([128, NJ, 48], f32d)
                nc.vector.tensor_tensor(dsb[:], ssb[:, :, 12:60],
                                        ssb[:, :, 0:48], Alu.subtract)

                osb = pp.tile([128, T, NJ, NO], f32d)
                r2 = pp.tile([128, NJ, NO], f32d)
                u = pp.tile([128, NJ, NO], f32d)
                dst = out_d.rearrange("t (p j) o -> p t j o", p=128)
                for t in range(1, T + 1):
                    k, p = (t - 1) // 6, (t - 1) % 6
                    if k == 0:
                        cv = ssb[:, :, 2 * p:2 * p + 2]
                    else:
                        cv = dsb[:, :, 12 * (k - 1) + 2 * p:
                                 12 * (k - 1) + 2 * p + 2]
                    if t == 1:
                        nc.vector.tensor_copy(osb[:, 0, :, :], cv)
                    else:
                        pm = osb[:, t - 2, :, :]
                        nc.vector.tensor_single_scalar(r2[:], pm, 1.0,
                                                       Alu.is_gt)
                        nc.vector.scalar_tensor_tensor(u[:], pm, float(BETA),
                                                       r2[:], Alu.mult,
                                                       Alu.subtract)
                        nc.vector.tensor_tensor(osb[:, t - 1, :, :], u[:], cv,
                                                Alu.add)
                    # stream each step's output as soon as it exists
                    nc.sync.dma_start(dst[:, t - 1, :, :],
                                      osb[:, t - 1, :, :])

    nc.compile()
    return nc


def _get_nc(tau, kf):
    key = "v2"
    if key not in _CACHE:
        _CACHE[key] = _build_nc(tau, kf)
    return _CACHE[key]


def kernel(x, W1, b1, W2, b2):
    global _LAST_RESULT, _LAST_IN_MAPS, _LAST_NC
    from concourse.bass_utils import run_bass_kernel_spmd

    x = np.ascontiguousarray(x, f32)
    sm, tau, kf, strong8 = _host_consts(np.asarray(W1, f32),
                                        np.asarray(b1, f32),
                                        np.asarray(W2, f32),
                                        np.asarray(b2, f32))
    nc = _get_nc(tau, kf)

    cols = np.arange(SH)
    perm = (cols % 128) * (SH // 128) + cols // 128
    ident = np.eye(128, dtype=f32)

    in_maps = []
    for i in range(NCORES):
        xs = x[i * SH:(i + 1) * SH][perm]          # [SH, 9]
        x_hi = _rnd11(xs)
        x_lo = _rnd11(xs - x_hi)
        xt3 = np.ones((28, SH), f32)
        xt3[0:9] = x_hi.T
        xt3[9:18] = x_lo.T
        xt3[18:26] = x_hi.T[strong8]
        in_maps.append({"xt3": xt3, "sm": sm, "ident": ident})

    _LAST_IN_MAPS = in_maps
    _LAST_NC = nc
    res = run_bass_kernel_spmd(nc, in_maps, list(range(NCORES)))
    _LAST_RESULT = res
    return np.concatenate([res.results[i]["out"] for i in range(NCORES)],
                          axis=1)


# revision 33
# speedup vs baseline: 1.0085x; 1.0085x over previous
"""Trainium2 Bass kernel for the SNN (snntorch Leaky, subtract-reset) forward.

Single fp32r matmul pass per step folds everything into one PE stream:
    P_t(h,b) accumulates  sum_s d_s*A(h,b) - sum_s c_s*spk_{s-1}
  where A = (W1 x + b1)/(1-beta), d_s = beta^-s (1-beta), c_s = beta^-s.
  Spike condition: mem_t > 1  <=>  P_t > tau_t = beta^-t  (scalar!).
Moving rows [128] = [spikes(100); x_hi(9); x_lo(9); x_hi strong8(8); 1; 1].
The x hi/lo/cross split delivers the theta decrement to ~22 bits despite
fp32r's ~11-bit ingest rounding. The spike subtract is exact via a two-
factor trick: the DVE compare writes spk in {0, K_t} (tensor_scalar
is_gt/mult) and the diag coefficient D_{t+1} is chosen so D_{t+1}*K_t ~
beta^-(t+1) to ~2^-22 (both factors 11-bit representable).
cur2 accumulates in PSUM rows 100:124 (6 pairs x 2 outs x hi/lo W2 split);
cumulative pair readouts after passes 7/13/19/25/26 are differenced in the
transposed post-phase, which also runs the cheap mem2 recurrence.
"""

import numpy as np

BETA = 0.95
T = 25
NPASS = 26
NI, NH, NO = 9, 100, 2
B = 65536
NCORES = 8
SH = B // NCORES          # 8192
RC = 2048
NR = SH // RC             # 4 rounds
NBLK = RC // 512
NJ = SH // 128            # 64 transpose blocks
f32 = np.float32
f64 = np.float64

_CACHE = {}
_LAST_RESULT = None
_LAST_IN_MAPS = None
_LAST_NC = None

RO_PASS = {7: 0, 13: 1, 19: 2, 25: 3, 26: 4}   # pass -> readout index


def _rnd11(v):
    """Round fp32 mantissa to 11 bits (half-up). Survives HW fp32r ingest."""
    v = np.asarray(v, f32)
    u = v.view(np.uint32).astype(np.uint64)
    u = (u + (np.uint64(1) << np.uint64(11))) >> np.uint64(12) << np.uint64(12)
    return (u & np.uint64(0xFFFFFFFF)).astype(np.uint32).view(f32)


def _find_DK(c):
    """11-bit pair (D, K) with D*K ~ c to ~2^-22."""
    best = (None, None, 1e9)
    for k in range(2048):
        K = 1.0 + k / 2048.0
        D = f64(_rnd11(f32(c / K)))
        e = abs(D * K - c)
        if e < best[2]:
            best = (D, K, e)
    return best[0], best[1]


def _host_consts(W1, b1, W2, b2):
    inv = 1.0 / (1.0 - f64(BETA))
    W1 = W1.astype(f64)
    b1 = b1.astype(f64)
    W2 = W2.astype(f64)
    b2 = b2.astype(f64)
    strong8 = np.argsort(-np.linalg.norm(W1, axis=0))[:8]

    # pair factorization: D_t * K_{t-1} ~ beta^-t  (t = 2..25)
    Kf = np.ones(T + 1, f64)          # Kf[t] scales spk_t (t = 1..25)
    Dg = np.zeros(NPASS + 1, f64)     # Dg[t] diag at pass t
    for t in range(2, T + 1):
        D, K = _find_DK(f64(BETA) ** -t)
        Dg[t] = D
        Kf[t - 1] = K
    tau = np.array([0.0] + [f32(f64(BETA) ** -t) for t in range(1, T + 1)],
                   f32)

    S = np.zeros((NPASS, 128, 124), f32)
    for t in range(1, NPASS + 1):
        s = S[t - 1]
        if 2 <= t:
            sp = t - 1                    # spike step consumed by this pass
            K = Kf[sp]
            if t <= T:
                np.fill_diagonal(s[0:NH, 0:NH], f32(-Dg[t]))
            p = (t - 2) % 6
            w2h = _rnd11((W2 / K).astype(f32))
            w2l = _rnd11(((W2 - w2h.astype(f64) * K) / K).astype(f32))
            for o in range(NO):
                s[0:NH, 100 + 2 * p + o] = w2h[o]
                s[0:NH, 112 + 2 * p + o] = w2l[o]
                s[126, 100 + 2 * p + o] = _rnd11(f32(b2[o]))
        if t <= T:
            d_t = f64(BETA) ** -t * (1.0 - f64(BETA))
            cf = d_t * W1 * inv                      # [NH, NI]
            chx = _rnd11(cf.astype(f32))
            clx = _rnd11((cf - chx.astype(f64)).astype(f32))
            ca = d_t * b1 * inv
            cah = _rnd11(ca.astype(f32))
            cal = _rnd11((ca - cah.astype(f64)).astype(f32))
            s[100:109, 0:NH] = chx.T
            s[109:118, 0:NH] = chx.T
            s[118:126, 0:NH] = clx.T[strong8]
            s[126, 0:NH] += cah
            s[127, 0:NH] = cal
    sm = S.transpose(1, 0, 2).reshape(128, NPASS * 124)
    return sm, tau, Kf.astype(f32), strong8


def _build_nc(tau, kf):
    import concourse.bass as bass
    import concourse.tile as tile
    from concourse import bacc, mybir

    f32d = mybir.dt.float32
    f32r = mybir.dt.float32r
    Copy = mybir.ActivationFunctionType.Copy
    Alu = mybir.AluOpType

    nc = bacc.Bacc("TRN2", target_bir_lowering=False, debug=False,
                   num_devices=NCORES)

    xt3_d = nc.dram_tensor("xt3", [28, SH], f32r, kind="ExternalInput").ap()
    sm_d = nc.dram_tensor("sm", [128, NPASS * 124], f32r,
                          kind="ExternalInput").ap()
    id_d = nc.dram_tensor("ident", [128, 128], f32d,
                          kind="ExternalInput").ap()
    out_d = nc.dram_tensor("out", [T, SH, NO], f32d,
                           kind="ExternalOutput").ap()

    with tile.TileContext(nc) as tc:
        with tc.tile_pool(name="const", bufs=1) as cp, \
             tc.tile_pool(name="ro", bufs=1) as rp:
            sm = cp.tile([128, NPASS * 124], f32r)
            ident = cp.tile([128, 128], f32d)
            # pass-1 slice first so the first matmuls start ~immediately
            nc.sync.dma_start(sm[:, 0:124], sm_d[:, 0:124])
            nc.sync.dma_start(sm[:, 124:3 * 124], sm_d[:, 124:3 * 124])
            nc.sync.dma_start(sm[:, 3 * 124:], sm_d[:, 3 * 124:])
            nc.sync.dma_start(ident[:], id_d[:])
            ro = rp.tile([120, SH], f32d)
            scr = [rp.tile([128, RC], f32d, name=f"scr{i}") for i in range(2)]

            with tc.tile_pool(name="spk", bufs=1) as kp:
                spks = [[kp.tile([128, RC], f32r, name=f"spk_{r}_{i}")
                         for i in range(2)] for r in range(NR)]
                for r in range(NR):
                    cs = slice(r * RC, (r + 1) * RC)
                    for i in range(2):
                        nc.sync.dma_start(spks[r][i][100:128, :],
                                          xt3_d[:, cs])
                    # pass 1 reads the parity-0 tile with zero spike coefs;
                    # garbage would still poison PSUM via 0*NaN
                    nc.vector.memset(spks[r][0][0:NH, :].bitcast(f32d), 0.0)

                ps_pool = tc.tile_pool(name="ps", bufs=2,
                                       space=bass.MemorySpace.PSUM)
                ps = ps_pool.__enter__()
                for ph in range(2):
                    rr = (2 * ph, 2 * ph + 1)
                    P = {r: ps.tile([128, RC], f32d, tag="P",
                                    name=f"P_{r}") for r in rr}
                    for t in range(1, NPASS + 1):
                        st = sm[:, (t - 1) * 124:t * 124]
                        for r in rr:
                            mv = spks[r][(t - 1) % 2]
                            for b in range(NBLK):
                                bs = slice(b * 512, (b + 1) * 512)
                                nc.tensor.matmul(
                                    P[r][0:124, bs], st, mv[:, bs],
                                    start=(t == 1), stop=True,
                                    skip_group_check=True)
                            if t <= T:
                                nc.vector.tensor_scalar(
                                    spks[r][t % 2][0:NH, :], P[r][0:NH, :],
                                    float(tau[t]), float(kf[t]),
                                    Alu.is_gt, Alu.mult)
                            if t in RO_PASS:
                                k = RO_PASS[t]
                                sc = scr[r % 2]
                                nc.scalar.activation(sc[96:128, :],
                                                     P[r][96:128, :], Copy,
                                                     bias=0.0, scale=1.0)
                                nc.sync.dma_start(
                                    ro[24 * k:24 * k + 24,
                                       r * RC:(r + 1) * RC],
                                    sc[100:124, :])
                ps_pool.__exit__(None, None, None)

            # ---- post: transpose, hi+lo, diff, mem2 recurrence, output ----
            with tc.tile_pool(name="post", bufs=1) as pp, \
                 tc.tile_pool(name="psT", bufs=1,
                              space=bass.MemorySpace.PSUM) as pt:
                tt = pp.tile([128, NJ, 120], f32d)
                for half in range(2):
                    # 128-col slots: each transpose output stays inside one
                    # 2KB PSUM bank (120-col slots would cross banks)
                    ptile = pt.tile([128, NJ // 2, 128], f32d, tag="tp",
                                    name=f"tp_{half}")
                    for jj in range(NJ // 2):
                        j = half * (NJ // 2) + jj
                        nc.tensor.transpose(
                            ptile[:, jj, 0:120],
                            ro[:, j * 128:(j + 1) * 128],
                            ident[0:120, 0:120])
                    nc.scalar.activation(
                        tt[:, half * (NJ // 2):(half + 1) * (NJ // 2), :],
                        ptile[:, :, 0:120], Copy, bias=0.0, scale=1.0)

                # s = hi + lo  -> [128, NJ, 60] (cols 12k + 2p + o)
                ssb = pp.tile([128, NJ, 60], f32d)
                tv = tt[:].rearrange("p j (k q) -> p (j k) q", q=24)
                sv = ssb[:].rearrange("p j (k q) -> p (j k) q", q=12)
                nc.vector.tensor_tensor(sv, tv[:, :, 0:12], tv[:, :, 12:24],
                                        Alu.add)
                # diff across consecutive readouts -> cur2 for t >= 7
                dsb = pp.tile([128, NJ, 48], f32d)
                nc.vector.tensor_tensor(dsb[:], ssb[:, :, 12:60],
                                        ssb[:, :, 0:48], Alu.subtract)

                osb = pp.tile([128, T, NJ, NO], f32d)
                r2 = pp.tile([128, NJ, NO], f32d)
                u = pp.tile([128, NJ, NO], f32d)
                dst = out_d.rearrange("t (p j) o -> p t j o", p=128)
                for t in range(1, T + 1):
                    k, p = (t - 1) // 6, (t - 1) % 6
                    if k == 0:
                        cv = ssb[:, :, 2 * p:2 * p + 2]
                    else:
                        cv = dsb[:, :, 12 * (k - 1) + 2 * p:
                                 12 * (k - 1) + 2 * p + 2]
                    if t == 1:
                        nc.vector.tensor_copy(osb[:, 0, :, :], cv)
                    else:
                        pm = osb[:, t - 2, :, :]
                        nc.vector.tensor_single_scalar(r2[:], pm, 1.0,
                                                       Alu.is_gt)
                        nc.vector.scalar_tensor_tensor(u[:], pm, float(BETA),
                                                       r2[:], Alu.mult,
                                                       Alu.subtract)
                        nc.vector.tensor_tensor(osb[:, t - 1, :, :], u[:], cv,
                                                Alu.add)
                    # stream each step's output as soon as it exists
                    nc.sync.dma_start(dst[:, t - 1, :, :],
                                      osb[:, t - 1, :, :])

    nc.compile()
    return nc


def _get_nc(tau, kf):
    key = "v2"
    if key not in _CACHE:
        _CACHE[key] = _build_nc(tau, kf)
    return _CACHE[key]


def kernel(x, W1, b1, W2, b2):
    global _LAST_RESULT, _LAST_IN_MAPS, _LAST_NC
    from concourse.bass_utils import run_bass_kernel_spmd

    x = np.ascontiguousarray(x, f32)
    sm, tau, kf, strong8 = _host_consts(np.asarray(W1, f32),
                                        np.asarray(b1, f32),
                                        np.asarray(W2, f32),
                                        np.asarray(b2, f32))
    nc = _get_nc(tau, kf)

    cols = np.arange(SH)
    perm = (cols % 128) * (SH // 128) + cols // 128
    ident = np.eye(128, dtype=f32)

    in_maps = []
    for i in range(NCORES):
        xs = x[i * SH:(i + 1) * SH][perm]          # [SH, 9]
        x_hi = _rnd11(xs)
        x_lo = _rnd11(xs - x_hi)
        xt3 = np.ones((28, SH), f32)
        xt3[0:9] = x_hi.T
        xt3[9:18] = x_lo.T
        xt3[18:26] = x_hi.T[strong8]
        in_maps.append({"xt3": xt3, "sm": sm, "ident": ident})

    _LAST_IN_MAPS = in_maps
    _LAST_NC = nc
    res = run_bass_kernel_spmd(nc, in_maps, list(range(NCORES)))
    _LAST_RESULT = res
    return np.concatenate([res.results[i]["out"] for i in range(NCORES)],
                          axis=1)
